# revision 14
# baseline (speedup 1.0000x reference)
"""Trainium2 Bass kernel for nn_CNNEncoder (gather -> lin1 -> conv1d -> maxpool -> MLP).

Strategy (v2)
-------------
Data-parallel over the 1024 = 64*16 sentences: 128 sentences per NeuronCore.

Host-side algebra: lin1 folds into the conv (Weff_k = W1 @ Wk), its bias into
the MLP bias (commutes with max-over-time).  Per core the 16384 tokens are
deduplicated to U <= 16384 unique ids, so indices fit int16 and the embedding
rows are served from a compact per-core table.

The table row is 512 bytes of interleaved fp8e4m3, built so that
`dma_gather(transpose=True)` (16-bit-unit transpose) lands channel-major
planes directly in SBUF -- no PE transposes at all:
  unit u (= 2 bytes), partition p = u%128, group g = u//128:
    byte0, g0/g1 : a  = fp8(e)            channels 0:256
    byte1, u 0:44: a  = fp8(e)            channels 256:300
    byte1, rest  : r8 = fp8(16*(e - a))   212 top-impact channels (e residual)

Conv = 15 fp8 DoubleRow matmuls (256-row contraction, 0.5 cyc/row) per
(o-chunk, 4-sentence block), PSUM-accumulated at a global 1024x scale:
  5x  a(g0,g1)  x fp8(1024*Weff_k)            (main)
  5x  b1 planes x [a-rows: fp8(1024*W); r-rows: fp8(64*W)]   (ch 256:300 + e-resid)
  5x  a(g0,g1)  x (1024*Weff_k - fp8(1024*Weff_k)) quantized (w residual)
Measured end-to-end rel-absmax error of this scheme: ~1.1e-2 (tol 2e-2).

DVE max over time -> cnn rows; tail MLP identical to v1 (f32r, biases folded
as ones-rows, 1/1024 folded into W2's cnn rows).
"""

import sys

sys.path.insert(0, "/opt/trn_rl_repo")

from contextlib import ExitStack

import numpy as np
import ml_dtypes

import concourse.bass as bass
import concourse.mybir as mybir
import concourse.tile as tile
from concourse import bacc, bass_utils
from concourse.library_config import mlp

F32 = mybir.dt.float32
F32R = mybir.dt.float32r
FP8 = mybir.dt.float8e4
I16 = mybir.dt.int16
FP8NP = ml_dtypes.float8_e4m3

VOCAB = 100000
D = 300
K = 5
L = 128          # tokens per sentence
NSENT = 1024     # total sentences
NCORES = 8
NS = NSENT // NCORES   # sentences per core = 128
SB = 4                 # sentences per block
NB = NS // SB          # 32 blocks
TP = L - K + 1         # 124 valid conv positions
GI = SB * L            # 512 gather indices per dma_gather (= 1 block)
UMAX = NS * L          # max unique tokens per core (16384)
ROWB = 512             # table row bytes (256 u16 units)
NPI = 212              # channels carrying an e-residual correction
CH = [(0, 128), (128, 256), (256, 300)]  # o-chunking of the 300 outputs
OCW = [128, 128, 64]   # weight-tile widths (dual-fp8 ldweights needs 64/128)
VT = (0, 2, 4)         # taps carrying the w-residual mirror (13-DR scheme)
NDR = 2 * K + len(VT)  # DoubleRow matmuls per (o-chunk, block)
WSEG = 2 * sum(OCW)    # 640 fp8 per (name, k) in the packed weight block
WTOT = NDR * WSEG      # packed weight bytes per partition
NWARM = 55             # PE warm-up transposes (hide prologue + p-state ramp)

CONV_DTYPE = "fp8dr"   # informational (printed by test.py)

_PROGRAM_CACHE = {}


def _build_program() -> bass.Bass:
    nc = bacc.Bacc(None, target_bir_lowering=False)

    tbl = nc.dram_tensor("tbl", [UMAX, ROWB], FP8, kind="ExternalInput")
    idx = nc.dram_tensor("idx", [128, NS * L // 16], I16, kind="ExternalInput")
    # all conv weights packed in one [128, WTOT] block: dual-fp8 ldweights
    # requires each [128, 2, ow] view to have contiguous planes, and one DMA
    # avoids 45 serialized HWDGE transfers stalling the PE prologue
    w_d = nc.dram_tensor("w", [128, WTOT], FP8, kind="ExternalInput")
    idn = nc.dram_tensor("idn", [128, 128], F32R, kind="ExternalInput")
    # tail weights with biases folded in as an extra contraction row:
    # w2cat = [W2 (600 rows, cnn rows /1024); b2eff] -> [601, D]
    w2cat = nc.dram_tensor("w2cat", [2 * D + 1, D], F32R, kind="ExternalInput")
    w3cat = nc.dram_tensor("w3cat", [D + 1, D], F32R, kind="ExternalInput")
    # mention_rep transposed, with a trailing all-ones row (drives the bias rows)
    m_t = nc.dram_tensor("mt", [D + 1, NS], F32R, kind="ExternalInput")
    out_d = nc.dram_tensor("out", [NS, D], F32, kind="ExternalOutput")

    with tile.TileContext(nc) as tc, ExitStack() as ctx:
        const = ctx.enter_context(tc.tile_pool(name="const", bufs=1))
        pspool = ctx.enter_context(tc.tile_pool(name="ps", bufs=6, space="PSUM"))

        nc.gpsimd.load_library(mlp)

        idx_sb = const.tile([128, NS * L // 16], I16)
        nc.sync.dma_start(out=idx_sb[:], in_=idx[:])

        w_sb = const.tile([128, WTOT], FP8)
        nc.sync.dma_start(out=w_sb[:], in_=w_d[:])
        wsb = {}
        off = 0
        for name in ("a", "r", "v"):
            for k in range(K):
                for oi, ow in enumerate(OCW):
                    wsb[(name, k, oi)] = w_sb[:, off : off + 2 * ow].rearrange(
                        "p (t m) -> p t m", t=2, m=ow
                    )
                    off += 2 * ow

        ident = const.tile([128, 128], F32R)
        nc.sync.dma_start(out=ident[:], in_=idn[:])

        # concat_T tiles [i-chunk, sent] for the tail contraction over the
        # 601-row [cnn(300); mention(300); ones] stack.  cnn rows are written
        # by the conv reduce_max; mention/ones rows DMA'd from m_t.
        W2CH = [(0, 128), (128, 256), (256, 384), (384, 512), (512, 601)]
        c_sb = [
            const.tile([c1 - c0, NS], F32R, tag=f"c_{c0}", name=f"c_{c0}")
            for c0, c1 in W2CH
        ]
        nc.sync.dma_start(out=c_sb[2][44:128, :], in_=m_t[0:84, :])
        nc.sync.dma_start(out=c_sb[3][:], in_=m_t[84:212, :])
        nc.sync.dma_start(out=c_sb[4][:], in_=m_t[212:301, :])

        w2cat_sb = []
        for c0, c1 in W2CH:
            t = const.tile([c1 - c0, D], F32R, tag=f"w2c_{c0}", name=f"w2c_{c0}")
            nc.sync.dma_start(out=t[:], in_=w2cat[c0:c1, :])
            w2cat_sb.append(t)

        JCH = [(0, 100), (100, 200), (200, 300)]
        w3cat_sb = []
        for j0, j1 in JCH:
            t = const.tile([j1 - j0, D], F32R, tag=f"w3c_{j0}", name=f"w3c_{j0}")
            nc.sync.dma_start(out=t[:], in_=w3cat[j0:j1, :])
            w3cat_sb.append(t)
        b3row_sb = const.tile([1, D], F32R)
        nc.sync.dma_start(out=b3row_sb[:], in_=w3cat[D : D + 1, :])
        ones_sb = const.tile([1, NS], F32R)
        nc.sync.dma_start(out=ones_sb[:], in_=m_t[D : D + 1, :])

        # ---- gathers: one per 4-sentence block, all issued up front ----
        gtiles = []
        for g in range(NB):
            gt = const.tile([128, 4, GI], FP8, tag=f"g{g}", name=f"g{g}")
            nc.gpsimd.dma_gather(
                gt[:],
                tbl[:],
                idx_sb[:, g * (GI // 16) : (g + 1) * (GI // 16)],
                GI,
                GI,
                ROWB,
                transpose=True,
            )
            gtiles.append(gt)

        # ---- conv: 15 DoubleRow matmuls per (block, o-chunk), then max ----
        DR = mybir.MatmulPerfMode.DoubleRow
        for b in range(NB):
            g5 = (
                gtiles[b][:]
                .rearrange("p f i -> p (f i)")
                .rearrange("p (g s t c) -> p g s t c", g=2, s=SB, t=L, c=2)
            )
            for oi, (o0, o1) in enumerate(CH):
                ps = pspool.tile([OCW[oi], SB, TP], F32, tag="ps")
                n = 0
                for name in ("a", "r", "v"):
                    byte = 1 if name == "r" else 0
                    for k in range(K):
                        nc.tensor.matmul(
                            out=ps[:],
                            lhsT=wsb[(name, k, oi)],
                            rhs=g5[:, :, :, k : k + TP, byte],
                            start=(n == 0),
                            stop=(n == 3 * K - 1),
                            perf_mode=DR,
                        )
                        n += 1
                nc.vector.tensor_reduce(
                    out=c_sb[oi][0 : o1 - o0, b * SB : (b + 1) * SB],
                    in_=ps[0 : o1 - o0],
                    axis=mybir.AxisListType.X,
                    op=mybir.AluOpType.max,
                )

        # ---- tail MLP, f32r full-rate (N=300), biases folded as ones-rows ----
        ps_h = pspool.tile([NS, D], F32, tag="ps")
        for c, (c0, c1) in enumerate(W2CH):
            nc.tensor.matmul(
                out=ps_h[:],
                lhsT=c_sb[c][:],
                rhs=w2cat_sb[c][:],
                start=(c == 0),
                stop=(c == len(W2CH) - 1),
            )
        h_sb = const.tile([NS, D], F32R)
        nc.scalar.activation(
            out=h_sb[:], in_=ps_h[:], func=mybir.ActivationFunctionType.Tanh
        )

        # transpose h -> h_T [j-chunk, s] for the second contraction
        ht_sb = []
        for jc, (j0, j1) in enumerate(JCH):
            ps_ht = pspool.tile([100, NS], F32R, tag="ps")
            nc.tensor.transpose(out=ps_ht[:], in_=h_sb[:, j0:j1], identity=ident[:])
            ht = const.tile([100, NS], F32R, tag=f"ht_{j0}", name=f"ht_{j0}")
            nc.scalar.copy(out=ht[:], in_=ps_ht[:])
            ht_sb.append(ht)

        # out[s, q] = sum_j h_T[j, s] * w3cat[j, q] + ones[s] * b3[q]
        ps_o = pspool.tile([NS, D], F32, tag="ps")
        for jc in range(3):
            nc.tensor.matmul(
                out=ps_o[:],
                lhsT=ht_sb[jc][:],
                rhs=w3cat_sb[jc][:],
                start=(jc == 0),
                stop=False,
            )
        nc.tensor.matmul(
            out=ps_o[:], lhsT=ones_sb[:], rhs=b3row_sb[:], start=False, stop=True
        )
        out_sb = const.tile([NS, D], F32)
        nc.scalar.copy(out=out_sb[:], in_=ps_o[:])
        nc.sync.dma_start(out=out_d[:], in_=out_sb[:])

    nc.finalize()
    return nc


def get_program() -> bass.Bass:
    if "p" not in _PROGRAM_CACHE:
        _PROGRAM_CACHE["p"] = _build_program()
    return _PROGRAM_CACHE["p"]


def _fp8(x):
    return np.asarray(x, dtype=np.float32).astype(FP8NP)


def _prepare_in_maps(inputs: dict) -> list[dict]:
    token_ids = np.asarray(inputs["token_ids"]).astype(np.int64)      # [1024, 128]
    mention = np.asarray(inputs["mention_rep"], dtype=np.float32).reshape(NSENT, D)
    emb = np.asarray(inputs["emb"], dtype=np.float32)
    W1 = np.asarray(inputs["W1"], dtype=np.float64)
    b1 = np.asarray(inputs["b1"], dtype=np.float64)
    conv_w = np.asarray(inputs["conv_w"], dtype=np.float64)           # [o, i, k]
    conv_b = np.asarray(inputs["conv_b"], dtype=np.float64)
    W2 = np.asarray(inputs["W2"], dtype=np.float64)                   # [2D, D]
    b2 = np.asarray(inputs["b2"], dtype=np.float64)
    W3 = np.asarray(inputs["W3"], dtype=np.float32)
    b3 = np.asarray(inputs["b3"], dtype=np.float32)

    Wk = conv_w.transpose(1, 0, 2)                                    # [i, o, k]
    weff = np.stack([W1 @ Wk[:, :, k] for k in range(K)])             # [k, i, o]
    beff = b1 @ Wk.sum(axis=2) + conv_b                               # [o]
    b2eff = b2 + beff @ W2[:D]                                        # [j]
    w2cat_h = np.concatenate([W2, b2eff[None, :]], axis=0).astype(np.float32)
    w2cat_h[:D] /= 1024.0                                             # cnn rows are 1024x
    w3cat_h = np.concatenate(
        [W3.astype(np.float64), np.asarray(inputs["b3"], np.float64)[None, :]], axis=0
    ).astype(np.float32)
    idn_h = np.eye(128, dtype=np.float32)

    # fp8 weight tiers (power-of-2 scales; products all land at 1024x)
    weff32 = weff.astype(np.float32)
    wa_q = _fp8(weff32 * 1024.0)                                      # [k, i, o]
    wv_q = _fp8(weff32 * 1024.0 - wa_q.astype(np.float32))
    wr_q = _fp8(weff32 * 64.0)

    # top-NPI channels by e-residual impact
    imp = (wa_q.astype(np.float32) ** 2).sum(axis=(0, 2))             # [i]
    PI = np.argsort(-imp)[:NPI].astype(np.int64)

    w_pack = np.zeros((128, WTOT), FP8NP)
    off = 0
    for name, wq in (("a", wa_q), ("r", wr_q), ("v", wv_q)):
        for k in range(K):
            for oi, (o0, o1) in enumerate(CH):
                ow = OCW[oi]
                seg = np.zeros((128, 2, ow), FP8NP)
                if name == "a":
                    seg[:, 0, : o1 - o0] = wa_q[k, 0:128, o0:o1]
                    seg[:, 1, : o1 - o0] = wa_q[k, 128:256, o0:o1]
                elif name == "r":
                    seg[0:44, 0, : o1 - o0] = wa_q[k, 256:300, o0:o1]
                    seg[44:128, 0, : o1 - o0] = wr_q[k, PI[0:84], o0:o1]
                    seg[:, 1, : o1 - o0] = wr_q[k, PI[84:212], o0:o1]
                else:
                    seg[:, 0, : o1 - o0] = wv_q[k, 0:128, o0:o1]
                    seg[:, 1, : o1 - o0] = wv_q[k, 128:256, o0:o1]
                w_pack[:, off : off + 2 * ow] = seg.reshape(128, 2 * ow)
                off += 2 * ow

    in_maps = []
    for c in range(NCORES):
        sl = slice(c * NS, (c + 1) * NS)
        toks = token_ids[sl]                                          # [128, 128]
        uniq, inv = np.unique(toks.ravel(), return_inverse=True)
        U = len(uniq)
        a8 = _fp8(emb[uniq])                                          # [U, 300]
        r8 = _fp8((emb[uniq] - a8.astype(np.float32)) * 16.0)
        tb = np.zeros((UMAX, ROWB), np.uint8)
        b0 = tb[:U, 0::2]
        b1v = tb[:U, 1::2]
        b0[:, 0:256] = a8.view(np.uint8)[:, 0:256]
        b1v[:, 0:44] = a8.view(np.uint8)[:, 256:300]
        b1v[:, 44:256] = r8.view(np.uint8)[:, PI]

        # idx layout: gather g's 512 ids in cols 32g:32g+32, wrapped over 16
        # partitions (token j -> [j%16, 32g + j//16]), replicated to 128.
        ids16 = inv.astype(np.int16).reshape(NB, GI)
        idx_h = np.zeros((128, NS * L // 16), np.int16)
        for g in range(NB):
            idx_h[0:16, g * 32 : (g + 1) * 32] = ids16[g].reshape(32, 16).T
        idx_h[16:128] = np.tile(idx_h[0:16], (7, 1))

        mt_h = np.ones((D + 1, NS), np.float32)
        mt_h[:D] = mention[sl].T
        in_maps.append(
            {
                "tbl": tb.view(FP8NP),
                "idx": idx_h,
                "idn": idn_h,
                "w2cat": w2cat_h,
                "w3cat": w3cat_h,
                "mt": mt_h,
                "w": w_pack,
            }
        )
    return in_maps


def run(inputs: dict, trace: bool = False, **kwargs):
    """Run the kernel; returns (output [1024, 300] f32, BassKernelResults)."""
    nc = get_program()
    in_maps = _prepare_in_maps(inputs)
    res = bass_utils.run_bass_kernel_spmd(
        nc, in_maps, core_ids=list(range(NCORES)), trace=trace, **kwargs
    )
    out = np.concatenate(
        [np.asarray(r["out"]) for r in res.results], axis=0
    ).astype(np.float32)
    return out, res


def kernel(**inputs) -> np.ndarray:
    out, _ = run(inputs)
    return out


# revision 26
# speedup vs baseline: 1.1234x; 1.1234x over previous
"""Trainium2 Bass kernel for nn_CNNEncoder (gather -> lin1 -> conv1d -> maxpool -> MLP).

Strategy (v2)
-------------
Data-parallel over the 1024 = 64*16 sentences: 128 sentences per NeuronCore.

Host-side algebra: lin1 folds into the conv (Weff_k = W1 @ Wk), its bias into
the MLP bias (commutes with max-over-time).  Per core the 16384 tokens are
deduplicated to U <= 16384 unique ids, so indices fit int16 and the embedding
rows are served from a compact per-core table.

The table row is 512 bytes of interleaved fp8e4m3, built so that
`dma_gather(transpose=True)` (16-bit-unit transpose) lands channel-major
planes directly in SBUF -- no PE transposes at all:
  unit u (= 2 bytes), partition p = u%128, group g = u//128:
    byte0, g0/g1 : a  = fp8(e)            channels 0:256
    byte1, u 0:44: a  = fp8(e)            channels 256:300
    byte1, rest  : r8 = fp8(16*(e - a))   212 top-impact channels (e residual)

Conv = 15 fp8 DoubleRow matmuls (256-row contraction, 0.5 cyc/row) per
(o-chunk, 4-sentence block), PSUM-accumulated at a global 1024x scale:
  5x  a(g0,g1)  x fp8(1024*Weff_k)            (main)
  5x  b1 planes x [a-rows: fp8(1024*W); r-rows: fp8(64*W)]   (ch 256:300 + e-resid)
  5x  a(g0,g1)  x (1024*Weff_k - fp8(1024*Weff_k)) quantized (w residual)
Measured end-to-end rel-absmax error of this scheme: ~1.1e-2 (tol 2e-2).

DVE max over time -> cnn rows; tail MLP identical to v1 (f32r, biases folded
as ones-rows, 1/1024 folded into W2's cnn rows).
"""

import sys

sys.path.insert(0, "/opt/trn_rl_repo")

from contextlib import ExitStack

import numpy as np
import ml_dtypes

import concourse.bass as bass
import concourse.mybir as mybir
import concourse.tile as tile
from concourse import bacc, bass_utils
from concourse.library_config import mlp

F32 = mybir.dt.float32
F32R = mybir.dt.float32r
FP8 = mybir.dt.float8e4
I16 = mybir.dt.int16
FP8NP = ml_dtypes.float8_e4m3

VOCAB = 100000
D = 300
K = 5
L = 128          # tokens per sentence
NSENT = 1024     # total sentences
NCORES = 8
NS = NSENT // NCORES   # sentences per core = 128
SB = 4                 # sentences per block
NB = NS // SB          # 32 blocks
TP = L - K + 1         # 124 valid conv positions
GI = SB * L            # 512 gather indices per dma_gather (= 1 block)
UMAX = NS * L          # max unique tokens per core (16384)
ROWB = 512             # table row bytes (256 u16 units)
NPI = 212              # channels carrying an e-residual correction
CH = [(0, 128), (128, 256), (256, 300)]  # o-chunking of the 300 outputs
OCW = [128, 128, 64]   # weight-tile widths (dual-fp8 ldweights needs 64/128)
VT = (0, 2, 4)         # taps carrying the w-residual mirror (13-DR scheme)
NDR = 2 * K + len(VT)  # DoubleRow matmuls per (o-chunk, block)
WSEG = 2 * sum(OCW)    # 640 fp8 per (name, k) in the packed weight block
WTOT = NDR * WSEG      # packed weight bytes per partition
NWARM = 55             # PE warm-up transposes (hide prologue + p-state ramp)

CONV_DTYPE = "fp8dr"   # informational (printed by test.py)

_PROGRAM_CACHE = {}


def _build_program() -> bass.Bass:
    nc = bacc.Bacc(None, target_bir_lowering=False)

    tbl = nc.dram_tensor("tbl", [UMAX, ROWB], FP8, kind="ExternalInput")
    idx = nc.dram_tensor("idx", [128, NS * L // 16], I16, kind="ExternalInput")
    # all conv weights packed in one [128, WTOT] block: dual-fp8 ldweights
    # requires each [128, 2, ow] view to have contiguous planes, and one DMA
    # avoids 45 serialized HWDGE transfers stalling the PE prologue
    w_d = nc.dram_tensor("w", [128, WTOT], FP8, kind="ExternalInput")
    idn = nc.dram_tensor("idn", [128, 128], F32R, kind="ExternalInput")
    # tail weights with biases folded in as an extra contraction row:
    # w2cat = [W2 (600 rows, cnn rows /1024); b2eff] -> [601, D]
    w2cat = nc.dram_tensor("w2cat", [2 * D + 1, D], F32R, kind="ExternalInput")
    w3cat = nc.dram_tensor("w3cat", [D + 1, D], F32R, kind="ExternalInput")
    # mention_rep transposed, with a trailing all-ones row (drives the bias rows)
    m_t = nc.dram_tensor("mt", [D + 1, NS], F32R, kind="ExternalInput")
    out_d = nc.dram_tensor("out", [NS, D], F32, kind="ExternalOutput")

    with tile.TileContext(nc) as tc, ExitStack() as ctx:
        const = ctx.enter_context(tc.tile_pool(name="const", bufs=1))
        pspool = ctx.enter_context(tc.tile_pool(name="ps", bufs=5, space="PSUM"))
        pstail = ctx.enter_context(tc.tile_pool(name="pst", bufs=1, space="PSUM"))

        nc.gpsimd.load_library(mlp)

        idx_sb = const.tile([128, NS * L // 16], I16)
        nc.sync.dma_start(out=idx_sb[:], in_=idx[:])

        w_sb = const.tile([128, WTOT], FP8)
        nc.sync.dma_start(out=w_sb[:], in_=w_d[:])
        wsb = {}
        off = 0
        for name in ("a", "r", "v"):
            taps = range(K) if name != "v" else VT
            for k in taps:
                for oi, ow in enumerate(OCW):
                    wsb[(name, k, oi)] = w_sb[:, off : off + 2 * ow].rearrange(
                        "p (t m) -> p t m", t=2, m=ow
                    )
                    off += 2 * ow

        # ---- gathers: one per 4-sentence block, all issued up front ----
        gtiles = []
        for g in range(NB):
            gt = const.tile([128, 4, GI], FP8, tag=f"g{g}", name=f"g{g}")
            nc.gpsimd.dma_gather(
                gt[:],
                tbl[:],
                idx_sb[:, g * (GI // 16) : (g + 1) * (GI // 16)],
                GI,
                GI,
                ROWB,
                transpose=True,
            )
            gtiles.append(gt)

        ident = const.tile([128, 128], F32R)
        nc.sync.dma_start(out=ident[:], in_=idn[:])

        # concat_T tiles [i-chunk, sent] for the tail contraction over the
        # 601-row [cnn(300); mention(300); ones] stack.  cnn rows are written
        # by the conv reduce_max; mention/ones rows DMA'd from m_t.
        W2CH = [(0, 128), (128, 256), (256, 384), (384, 512), (512, 601)]
        c_sb = [
            const.tile([c1 - c0, NS], F32R, tag=f"c_{c0}", name=f"c_{c0}")
            for c0, c1 in W2CH
        ]
        nc.sync.dma_start(out=c_sb[2][44:128, :], in_=m_t[0:84, :])
        nc.sync.dma_start(out=c_sb[3][:], in_=m_t[84:212, :])
        nc.sync.dma_start(out=c_sb[4][:], in_=m_t[212:301, :])

        w2cat_sb = []
        for c0, c1 in W2CH:
            t = const.tile([c1 - c0, D], F32R, tag=f"w2c_{c0}", name=f"w2c_{c0}")
            nc.sync.dma_start(out=t[:], in_=w2cat[c0:c1, :])
            w2cat_sb.append(t)

        JCH = [(0, 100), (100, 200), (200, 300)]
        w3cat_sb = []
        for j0, j1 in JCH:
            t = const.tile([j1 - j0, D], F32R, tag=f"w3c_{j0}", name=f"w3c_{j0}")
            nc.sync.dma_start(out=t[:], in_=w3cat[j0:j1, :])
            w3cat_sb.append(t)
        b3row_sb = const.tile([1, D], F32R)
        nc.sync.dma_start(out=b3row_sb[:], in_=w3cat[D : D + 1, :])
        ones_sb = const.tile([1, NS], F32R)
        nc.sync.dma_start(out=ones_sb[:], in_=m_t[D : D + 1, :])

        # tail tiles shared by both sentence-halves
        ht_sb = [
            const.tile([100, NS], F32R, tag=f"ht_{j0}", name=f"ht_{j0}")
            for j0, j1 in JCH
        ]

        def tail_half(h: int):
            """MLP tail for sentences 64h:64h+64 (half 0 runs under the conv
            of blocks 16-31; half 1 at the end)."""
            hs = slice(64 * h, 64 * (h + 1))
            ps_h = pstail.tile([64, D], F32, tag="ps_tail")
            # mention chunks (3, 4) first: their inputs are ready long before
            # the cnn chunks, hiding the final DVE reduce drain
            corder = [3, 4, 0, 1, 2]
            for i, c in enumerate(corder):
                nc.tensor.matmul(
                    out=ps_h[:],
                    lhsT=c_sb[c][:, hs],
                    rhs=w2cat_sb[c][:],
                    start=(i == 0),
                    stop=(i == len(corder) - 1),
                )
            h_sb = const.tile([64, D], F32R, tag=f"h{h}")
            nc.scalar.activation(
                out=h_sb[:], in_=ps_h[:], func=mybir.ActivationFunctionType.Tanh
            )
            for jc, (j0, j1) in enumerate(JCH):
                ps_ht = pstail.tile([100, 64], F32R, tag="ps_tail2")
                nc.tensor.transpose(
                    out=ps_ht[:], in_=h_sb[:, j0:j1], identity=ident[0:64, 0:64]
                )
                nc.scalar.copy(out=ht_sb[jc][:, hs], in_=ps_ht[:])
            ps_o = pstail.tile([64, D], F32, tag="ps_tail")
            for jc in range(3):
                nc.tensor.matmul(
                    out=ps_o[:],
                    lhsT=ht_sb[jc][:, hs],
                    rhs=w3cat_sb[jc][:],
                    start=(jc == 0),
                    stop=False,
                )
            nc.tensor.matmul(
                out=ps_o[:], lhsT=ones_sb[:, hs], rhs=b3row_sb[:],
                start=False, stop=True,
            )
            out_sb = const.tile([64, D], F32, tag=f"os{h}")
            nc.scalar.copy(out=out_sb[:], in_=ps_o[:])
            nc.sync.dma_start(out=out_d[hs, :], in_=out_sb[:])

        # ---- conv: NDR DoubleRow matmuls per (block, o-chunk), then max ----
        DR = mybir.MatmulPerfMode.DoubleRow
        for b in range(NB):
            if b == NB // 2 + 1:
                tail_half(0)
            g5 = (
                gtiles[b][:]
                .rearrange("p f i -> p (f i)")
                .rearrange("p (g s t c) -> p g s t c", g=2, s=SB, t=L, c=2)
            )
            for oi, (o0, o1) in enumerate(CH):
                ps = pspool.tile([OCW[oi], SB, TP], F32, tag="ps")
                n = 0
                for name in ("a", "r", "v"):
                    byte = 1 if name == "r" else 0
                    taps = range(K) if name != "v" else VT
                    for k in taps:
                        nc.tensor.matmul(
                            out=ps[:],
                            lhsT=wsb[(name, k, oi)],
                            rhs=g5[:, :, :, k : k + TP, byte],
                            start=(n == 0),
                            stop=(n == NDR - 1),
                            perf_mode=DR,
                        )
                        n += 1
                nc.vector.tensor_reduce(
                    out=c_sb[oi][0 : o1 - o0, b * SB : (b + 1) * SB],
                    in_=ps[0 : o1 - o0],
                    axis=mybir.AxisListType.X,
                    op=mybir.AluOpType.max,
                )

        tail_half(1)

    nc.finalize()
    return nc


def get_program() -> bass.Bass:
    if "p" not in _PROGRAM_CACHE:
        _PROGRAM_CACHE["p"] = _build_program()
    return _PROGRAM_CACHE["p"]


def _fp8(x):
    return np.asarray(x, dtype=np.float32).astype(FP8NP)


def _prepare_in_maps(inputs: dict) -> list[dict]:
    token_ids = np.asarray(inputs["token_ids"]).astype(np.int64)      # [1024, 128]
    mention = np.asarray(inputs["mention_rep"], dtype=np.float32).reshape(NSENT, D)
    emb = np.asarray(inputs["emb"], dtype=np.float32)
    W1 = np.asarray(inputs["W1"], dtype=np.float64)
    b1 = np.asarray(inputs["b1"], dtype=np.float64)
    conv_w = np.asarray(inputs["conv_w"], dtype=np.float64)           # [o, i, k]
    conv_b = np.asarray(inputs["conv_b"], dtype=np.float64)
    W2 = np.asarray(inputs["W2"], dtype=np.float64)                   # [2D, D]
    b2 = np.asarray(inputs["b2"], dtype=np.float64)
    W3 = np.asarray(inputs["W3"], dtype=np.float32)
    b3 = np.asarray(inputs["b3"], dtype=np.float32)

    Wk = conv_w.transpose(1, 0, 2)                                    # [i, o, k]
    weff = np.stack([W1 @ Wk[:, :, k] for k in range(K)])             # [k, i, o]
    beff = b1 @ Wk.sum(axis=2) + conv_b                               # [o]
    b2eff = b2 + beff @ W2[:D]                                        # [j]
    w2cat_h = np.concatenate([W2, b2eff[None, :]], axis=0).astype(np.float32)
    w2cat_h[:D] /= 1024.0                                             # cnn rows are 1024x
    w3cat_h = np.concatenate(
        [W3.astype(np.float64), np.asarray(inputs["b3"], np.float64)[None, :]], axis=0
    ).astype(np.float32)
    idn_h = np.eye(128, dtype=np.float32)

    # fp8 weight tiers (power-of-2 scales; products all land at 1024x)
    weff32 = weff.astype(np.float32)
    wa_q = _fp8(weff32 * 1024.0)                                      # [k, i, o]
    wv_q = _fp8(weff32 * 1024.0 - wa_q.astype(np.float32))
    wr_q = _fp8(weff32 * 64.0)

    # top-NPI channels by e-residual impact
    imp = (wa_q.astype(np.float32) ** 2).sum(axis=(0, 2))             # [i]
    PI = np.argsort(-imp)[:NPI].astype(np.int64)

    w_pack = np.zeros((128, WTOT), FP8NP)
    off = 0
    for name, wq in (("a", wa_q), ("r", wr_q), ("v", wv_q)):
        for k in range(K) if name != "v" else VT:
            for oi, (o0, o1) in enumerate(CH):
                ow = OCW[oi]
                seg = np.zeros((128, 2, ow), FP8NP)
                if name == "a":
                    seg[:, 0, : o1 - o0] = wa_q[k, 0:128, o0:o1]
                    seg[:, 1, : o1 - o0] = wa_q[k, 128:256, o0:o1]
                elif name == "r":
                    seg[0:44, 0, : o1 - o0] = wa_q[k, 256:300, o0:o1]
                    seg[44:128, 0, : o1 - o0] = wr_q[k, PI[0:84], o0:o1]
                    seg[:, 1, : o1 - o0] = wr_q[k, PI[84:212], o0:o1]
                else:
                    seg[:, 0, : o1 - o0] = wv_q[k, 0:128, o0:o1]
                    seg[:, 1, : o1 - o0] = wv_q[k, 128:256, o0:o1]
                w_pack[:, off : off + 2 * ow] = seg.reshape(128, 2 * ow)
                off += 2 * ow

    in_maps = []
    for c in range(NCORES):
        sl = slice(c * NS, (c + 1) * NS)
        toks = token_ids[sl]                                          # [128, 128]
        uniq, inv = np.unique(toks.ravel(), return_inverse=True)
        U = len(uniq)
        a8 = _fp8(emb[uniq])                                          # [U, 300]
        r8 = _fp8((emb[uniq] - a8.astype(np.float32)) * 16.0)
        tb = np.zeros((UMAX, ROWB), np.uint8)
        b0 = tb[:U, 0::2]
        b1v = tb[:U, 1::2]
        b0[:, 0:256] = a8.view(np.uint8)[:, 0:256]
        b1v[:, 0:44] = a8.view(np.uint8)[:, 256:300]
        b1v[:, 44:256] = r8.view(np.uint8)[:, PI]

        # idx layout: gather g's 512 ids in cols 32g:32g+32, wrapped over 16
        # partitions (token j -> [j%16, 32g + j//16]), replicated to 128.
        ids16 = inv.astype(np.int16).reshape(NB, GI)
        idx_h = np.zeros((128, NS * L // 16), np.int16)
        for g in range(NB):
            idx_h[0:16, g * 32 : (g + 1) * 32] = ids16[g].reshape(32, 16).T
        idx_h[16:128] = np.tile(idx_h[0:16], (7, 1))

        mt_h = np.ones((D + 1, NS), np.float32)
        mt_h[:D] = mention[sl].T
        in_maps.append(
            {
                "tbl": tb.view(FP8NP),
                "idx": idx_h,
                "idn": idn_h,
                "w2cat": w2cat_h,
                "w3cat": w3cat_h,
                "mt": mt_h,
                "w": w_pack,
            }
        )
    return in_maps


def run(inputs: dict, trace: bool = False, **kwargs):
    """Run the kernel; returns (output [1024, 300] f32, BassKernelResults)."""
    nc = get_program()
    in_maps = _prepare_in_maps(inputs)
    res = bass_utils.run_bass_kernel_spmd(
        nc, in_maps, core_ids=list(range(NCORES)), trace=trace, **kwargs
    )
    out = np.concatenate(
        [np.asarray(r["out"]) for r in res.results], axis=0
    ).astype(np.float32)
    return out, res


def kernel(**inputs) -> np.ndarray:
    out, _ = run(inputs)
    return out
